# revision 39
# baseline (speedup 1.0000x reference)
"""Trainium2 Bass kernel for a dense transformer block (2x2048x1024, 16 heads,
MLP hidden 4096), SPMD over 8 NeuronCores.

Sharding: attention is head-sharded (2 heads per core, both batches); an
AllToAll converts head shards into token shards, after which proj/LN2/MLP run
on 512 tokens per core. All matmuls are bf16 with fp32 PSUM accumulation.
LayerNorm is folded into the QKV matmul epilogue: with P = W_g1 @ x^T,
qkv^T = P * istd + (-mu*istd) * rowsum(W_g1), so the normalized activations
are never materialized for the matmul path. Softmax skips max-subtraction
(scores are bounded ~|4| for this problem) and gets its denominator from an
appended ones-column in the AV matmul.

Optimizations over the original baseline:
- fc1/fc2/proj weights stored host-side in [128, N] layouts with 2KB
  contiguous partition lines and loaded with a handful of batched DMAs that
  overlap the AllToAll wait (wp+fc1 fully resident, fc2 triple-buffered).
- Attention scores land in 3-bank fp32 PSUM group tiles so each ACT exp
  instruction covers 1536 elements/lane; per-block softmax epilogues
  (reciprocal broadcast matmuls + scaling) are deferred to a tail after the
  whole attention loop so the PE instruction queue never stalls on the
  reciprocal chain and HAM stays un-throttled.
- LayerNorm istd uses exp(-0.5*ln(var+eps)) on the ACT engine, which shares
  one activation table set (natural_log_exp) with the softmax exp; squares
  ride half on ACT, half on DVE.
- The AllToAll payload is quantized to fp8e4m3 (error budget allows it) and
  widened back to bf16 on the receiving side; x-block/weight/const DMAs are
  batched into multi-dim single instructions to cut sync-engine issue time.
"""

from contextlib import ExitStack

import numpy as np
import ml_dtypes

import concourse.bass as bass
import concourse.mybir as mybir
from concourse import tile
from concourse.bass_utils import run_bass_kernel_spmd
from concourse.vector_clock import ScopedClock

F32 = mybir.dt.float32
BF16 = mybir.dt.bfloat16
F8 = mybir.dt.float8e4
AF = mybir.ActivationFunctionType
OP = mybir.AluOpType

N_CORES = 8
B, L, D = 2, 2048, 1024
NH, HD = 16, 64
HID = 4096
T = B * L            # 4096 tokens total
TOK = T // N_CORES   # 512 tokens per core after the A2A
KT = D // 128        # 8 k-tiles over the model dim
NJ = L // 128        # 16 j-tiles per batch
NI = L // TOK        # 4 i-blocks per batch
EPS = 1e-6
GROUPS = [list(range(N_CORES))]
NH2 = HID // 128     # 32 hidden tiles


class ChunkedDrainTileContext(tile.TileContext):
    """This walrus build only accepts one explicit sem wait per CTRL
    instruction; split the kernel-tail drain's waits across a chain."""

    MAX_WAITS = 1

    def _drain_and_barrier(self, tick_clock, wait_clock):
        drain_inst = self.nc.sync.drain()
        wait_clock.add_sem_waits(
            drain_inst.ins, ScopedClock({None: tick_clock.global_clock})
        )
        si = drain_inst.ins.sync_info
        if si is not None and len(si.on_wait) > self.MAX_WAITS:
            waits = list(si.on_wait)
            si.on_wait = waits[: self.MAX_WAITS]
            for i in range(self.MAX_WAITS, len(waits), self.MAX_WAITS):
                extra = self.nc.sync.drain()
                extra.ins.sync_info = mybir.SyncInfo(
                    on_wait=waits[i : i + self.MAX_WAITS], on_update=[]
                )
        self.nc.all_engine_barrier()
        assert self.sems is not None
        popped = self.nc._tile_sem_poison_stack.pop()
        assert popped is self._sem_poison
        self.nc.clear_and_free_semaphores(list(self.sems.allocated().values()))
        self.nc.all_engine_barrier()


def _split_multi_waits(nc):
    """This walrus build accepts at most one sync wait per instruction; hoist
    extra waits onto preceding same-engine NoOps."""
    n = 0
    for fn in nc.m.functions:
        for bb in fn.blocks:
            insts = bb.instructions
            new = []
            for ins in insts:
                si = ins.sync_info
                if si is not None and len(si.on_wait) > 1:
                    waits = list(si.on_wait)
                    si.on_wait = [waits[-1]]
                    for w in waits[:-1]:
                        n += 1
                        nop = mybir.InstNoOp(
                            name=f"waitsplit-{n}",
                            sync_info=mybir.SyncInfo(on_wait=[w], on_update=[]),
                            bass_nofuse=True,
                            engine=ins.engine,
                        )
                        nc.register_instruction(nop)
                        new.append(nop)
                new.append(ins)
            if len(new) != len(insts):
                bb.instructions = new
    return n


def _ln_stats(nc, pool, psum_s, psum_q, eps_ap):
    """From replicated column sums / sums-of-squares, produce replicated
    A = 1/std and B = -mu/std tiles, all [128, TOK] f32."""
    inv_d = 1.0 / D
    mu = pool.tile([128, TOK], F32, tag="mu", name="mu")
    nc.scalar.mul(mu[:], psum_s[:], inv_d)
    musq = pool.tile([128, TOK], F32, tag="musq", name="musq")
    nc.scalar.activation(musq[:], mu[:], AF.Square)
    var = pool.tile([128, TOK], F32, tag="var", name="var")
    nc.vector.scalar_tensor_tensor(
        var[:], psum_q[:], inv_d, musq[:], OP.mult, OP.subtract
    )
    # 1/sqrt(var+eps) = exp(-0.5*ln(var+eps)) -- keeps everything on ACT in
    # the same table set as the softmax exp (natural_log_exp_and_others).
    sv = pool.tile([128, TOK], F32, tag="sv", name="sv")
    nc.scalar.activation(sv[:], var[:], AF.Ln, bias=eps_ap)
    a_t = pool.tile([128, TOK], F32, tag="a_t", name="a_t")
    nc.scalar.activation(a_t[:], sv[:], AF.Exp, scale=-0.5)
    b_t = pool.tile([128, TOK], F32, tag="b_t", name="b_t")
    nc.vector.scalar_tensor_tensor(b_t[:], mu[:], -1.0, a_t[:], OP.mult, OP.mult)
    return a_t, b_t


def build_program():
    nc = bass.Bass(
        "TRN2", target_bir_lowering=False, debug=False, num_devices=N_CORES
    )

    xT = nc.dram_tensor("xT", [D, T], BF16, kind="ExternalInput")
    xres = nc.dram_tensor("xres", [D, TOK], F32, kind="ExternalInput")
    wqT = nc.dram_tensor("wqT", [D, 128], BF16, kind="ExternalInput")
    wkT = nc.dram_tensor("wkT", [D, 128], BF16, kind="ExternalInput")
    wvT = nc.dram_tensor("wvT", [D, 128], BF16, kind="ExternalInput")
    wqs = nc.dram_tensor("wqs", [128, 1], F32, kind="ExternalInput")
    wks = nc.dram_tensor("wks", [128, 1], F32, kind="ExternalInput")
    wvs = nc.dram_tensor("wvs", [128, 1], F32, kind="ExternalInput")
    wpT = nc.dram_tensor("wpT", [D, D], BF16, kind="ExternalInput")
    fc1T = nc.dram_tensor("fc1T", [128, NH2 * D], BF16, kind="ExternalInput")
    fc2T = nc.dram_tensor("fc2T", [128, KT * HID], BF16, kind="ExternalInput")
    g1c = nc.dram_tensor("g1c", [D, 1], F32, kind="ExternalInput")
    g2c = nc.dram_tensor("g2c", [D, 1], F32, kind="ExternalInput")
    ident = nc.dram_tensor("ident", [128, 128], BF16, kind="ExternalInput")
    outT = nc.dram_tensor("outT", [D, TOK], F32, kind="ExternalOutput")

    with ChunkedDrainTileContext(nc) as tc, ExitStack() as outer:
        p_const = outer.enter_context(tc.tile_pool(name="const", bufs=1))
        p_dram = outer.enter_context(tc.tile_pool(name="dram", bufs=1, space="DRAM"))

        ones = p_const.tile([128, 128], BF16, tag="ones", name="ones")
        nc.gpsimd.memset(ones[:], 1.0)
        idt = p_const.tile([128, 128], BF16, tag="idt", name="idt")
        g1t = p_const.tile([128, KT], F32, tag="g1t", name="g1t")
        g2t = p_const.tile([128, KT], F32, tag="g2t", name="g2t")
        epst = p_const.tile([128, 1], F32, tag="epst", name="epst")
        nc.gpsimd.memset(epst[:], EPS)

        def load_consts():
            # deferred so the first x-block DMA wins the sync queue at startup
            nc.sync.dma_start(idt[:], ident[:])
            nc.sync.dma_start(
                g1t[:].rearrange("p (k c) -> p k c", k=KT),
                g1c[:].rearrange("(k p) c -> p k c", p=128),
            )
            nc.sync.dma_start(
                g2t[:].rearrange("p (k c) -> p k c", k=KT),
                g2c[:].rearrange("(k p) c -> p k c", p=128),
            )

        send = p_dram.tile([T // 4, TOK], F8, tag="send", name="send")
        recv = p_dram.tile([T // 4, TOK], F8, tag="recv", name="recv")

        with ExitStack() as qscope:
            p_qkvT = qscope.enter_context(tc.tile_pool(name="qkvT", bufs=1))
            qTt = p_qkvT.tile([128, T], BF16, tag="qT", name="qT")
            kTt = p_qkvT.tile([128, T], BF16, tag="kT", name="kT")
            vTt = p_qkvT.tile([128, T], BF16, tag="vT", name="vT")

            # ======== Phase 1: LN1 stats + QKV ========
            with (
                tc.tile_pool(name="xtb", bufs=3) as p_xtb,
                tc.tile_pool(name="wqkv", bufs=1) as p_wqkv,
                tc.tile_pool(name="sq", bufs=2) as p_sq,
                tc.tile_pool(name="stats", bufs=2) as p_stats,
                tc.tile_pool(name="fix", bufs=4) as p_fix,
                tc.tile_pool(name="ps1", bufs=2, space="PSUM") as ps1,
                tc.tile_pool(name="ps1b", bufs=2, space="PSUM") as ps1b,
            ):
                xa_tiles = {}

                def load_xa(tb):
                    sl = slice(TOK * tb, TOK * (tb + 1))
                    # one batched DMA per token block: [128, KT*TOK] where
                    # column group k holds xT[128k:128(k+1), sl]
                    xa = p_xtb.tile([128, KT * TOK], BF16, tag="xtb", name="xtb")
                    nc.sync.dma_start(
                        xa[:].rearrange("p (k t) -> p k t", k=KT),
                        xT[:, sl].rearrange("(k p) t -> p k t", p=128),
                    )
                    xa_tiles[tb] = xa

                load_xa(0)
                wqkv = []
                for src, name in ((wqT, "wq"), (wkT, "wk"), (wvT, "wv")):
                    t = p_wqkv.tile([128, D], BF16, tag=name, name=name)
                    nc.sync.dma_start(
                        t[:].rearrange("p (k c) -> p k c", k=KT),
                        src[:].rearrange("(k p) c -> p k c", p=128),
                    )
                    wqkv.append(t)
                wqsum = p_wqkv.tile([128, 1], F32, tag="wqsum", name="wqsum")
                nc.sync.dma_start(wqsum[:], wqs[:])
                wksum = p_wqkv.tile([128, 1], F32, tag="wksum", name="wksum")
                nc.sync.dma_start(wksum[:], wks[:])
                wvsum = p_wqkv.tile([128, 1], F32, tag="wvsum", name="wvsum")
                nc.sync.dma_start(wvsum[:], wvs[:])
                load_xa(1)
                load_consts()

                for tb in range(T // TOK):
                    sl = slice(TOK * tb, TOK * (tb + 1))
                    if tb + 2 < T // TOK:
                        load_xa(tb + 2)
                    xa = xa_tiles.pop(tb)
                    xtb = [xa[:, TOK * k : TOK * (k + 1)] for k in range(KT)]
                    sqa = p_sq.tile([128, KT * TOK], BF16, tag="sq", name="sq")
                    half = KT * TOK // 2
                    nc.scalar.activation(sqa[:, 0:half], xa[:, 0:half], AF.Square)
                    nc.vector.scalar_tensor_tensor(
                        sqa[:, half:], xa[:, half:], 1.0, xa[:, half:],
                        OP.mult, OP.mult,
                    )
                    sq = [sqa[:, TOK * k : TOK * (k + 1)] for k in range(KT)]
                    psum_s = ps1.tile([128, TOK], F32, tag="ps_s", name="ps_s")
                    psum_q = ps1.tile([128, TOK], F32, tag="ps_q", name="ps_q")
                    for k in range(KT):
                        nc.tensor.matmul(
                            psum_s[:], ones[:], xtb[k],
                            start=(k == 0), stop=(k == KT - 1),
                        )
                    for k in range(KT):
                        nc.tensor.matmul(
                            psum_q[:], ones[:], sq[k],
                            start=(k == 0), stop=(k == KT - 1),
                        )
                    a_t, b_t = _ln_stats(nc, p_stats, psum_s, psum_q, epst[:])

                    for dst, w, wsum in (
                        (qTt, wqkv[0], wqsum),
                        (kTt, wqkv[1], wksum),
                        (vTt, wqkv[2], wvsum),
                    ):
                        pm = ps1b.tile([128, TOK], F32, tag="ps_qkv", name="ps_qkv")
                        for k in range(KT):
                            nc.tensor.matmul(
                                pm[:], w[:, 128 * k : 128 * (k + 1)], xtb[k],
                                start=(k == 0), stop=(k == KT - 1),
                            )
                        u = p_fix.tile([128, TOK], F32, tag="fixu", name="fixu")
                        nc.vector.scalar_tensor_tensor(
                            u[:], pm[:], 1.0, a_t[:], OP.mult, OP.mult
                        )
                        nc.vector.scalar_tensor_tensor(
                            dst[:, sl], b_t[:], wsum[:], u[:], OP.mult, OP.add
                        )

            # ======== Phase 2: attention ========
            with (
                tc.tile_pool(name="vones", bufs=1) as p_vones,
                tc.tile_pool(name="es", bufs=4) as p_es,
                tc.tile_pool(name="attn", bufs=3) as p_attn,
                tc.tile_pool(name="ohs", bufs=2) as p_ohs,
            ):
                vones = []
                with tc.tile_pool(name="psv", bufs=2, space="PSUM") as psv:
                    for t in range(T // 128):
                        pm = psv.tile([128, 128], BF16, tag="ps_vt", name="ps_vt")
                        nc.tensor.transpose(
                            pm[:], vTt[:, 128 * t : 128 * (t + 1)], idt[:]
                        )
                        vo = p_vones.tile(
                            [128, 130], BF16, tag=f"vo{t}", name=f"vo{t}"
                        )
                        nc.gpsimd.memset(vo[:], 1.0)
                        nc.vector.tensor_copy(vo[:, 0:64], pm[:, 0:64])
                        nc.vector.tensor_copy(vo[:, 65:129], pm[:, 64:128])
                        vones.append(vo)

                finished = []
                with (
                    tc.tile_pool(name="pss", bufs=2, space="PSUM") as pss,
                    tc.tile_pool(name="pso", bufs=1, space="PSUM") as pso,
                ):
                    for b in range(B):
                        for i in range(NI):
                            isl = slice(b * L + TOK * i, b * L + TOK * (i + 1))
                            po0 = pso.tile([65, TOK], F32, tag="po0", name="po0")
                            po1 = pso.tile([65, TOK], F32, tag="po1", name="po1")
                            po = (po0, po1)
                            # 32 score tiles (j-major, head-minor), exp'd in
                            # groups of 3 from a 3-bank PSUM tile
                            for g in range(11):
                                nslot = 3 if g < 10 else 2
                                sg = pss.tile(
                                    [128, 1536], F32, tag="sg", name="sg"
                                )
                                for u_ in range(nslot):
                                    s = 3 * g + u_
                                    j, h = s // 2, s % 2
                                    jsl = slice(
                                        b * L + 128 * j, b * L + 128 * (j + 1)
                                    )
                                    nc.tensor.matmul(
                                        sg[:, 512 * u_ : 512 * (u_ + 1)],
                                        kTt[64 * h : 64 * (h + 1), jsl],
                                        qTt[64 * h : 64 * (h + 1), isl],
                                        start=True, stop=True,
                                        tile_position=(64 * h, 0),
                                    )
                                es = p_es.tile(
                                    [128, 1536], BF16, tag="es", name="es"
                                )
                                nc.scalar.activation(
                                    es[:, 0 : 512 * nslot], sg[:, 0 : 512 * nslot],
                                    AF.Exp,
                                )
                                for u_ in range(nslot):
                                    s = 3 * g + u_
                                    j, h = s // 2, s % 2
                                    vo = vones[b * NJ + j]
                                    nc.tensor.matmul(
                                        po[h][:],
                                        vo[:, 65 * h : 65 * (h + 1)],
                                        es[:, 512 * u_ : 512 * (u_ + 1)],
                                        start=(s == h), stop=(s == 30 + h),
                                    )
                            # copy AV results to SBUF promptly (frees the po
                            # PSUM banks for the next block's accumulation)
                            pob = p_attn.tile(
                                [65, 2 * TOK], F32, tag="pob", name="pob", bufs=8
                            )
                            nc.vector.tensor_copy(pob[:, 0:TOK], po0[:])
                            nc.vector.tensor_copy(pob[:, TOK : 2 * TOK], po1[:])
                            # softmax denominators sit in row 64
                            rc = p_attn.tile(
                                [65, 2 * TOK], F32, tag="rc", name="rc", bufs=2
                            )
                            nc.vector.reciprocal(rc[64:65, :], pob[64:65, :])
                            rcb = p_attn.tile(
                                [65, 2 * TOK], BF16, tag="rcb", name="rcb", bufs=8
                            )
                            nc.vector.tensor_copy(rcb[64:65, :], rc[64:65, :])
                            finished.append((pob, rcb, b * NI + i))

                # epilogue tail: broadcast 1/denominator across partitions via
                # PE, scale, and ship each block to its destination core
                with tc.tile_pool(name="psr", bufs=1, space="PSUM") as psr:
                    for pob, rcb, s_idx in finished:
                        pr0 = psr.tile([64, TOK], F32, tag="pr0", name="pr0")
                        pr1 = psr.tile([64, TOK], F32, tag="pr1", name="pr1")
                        nc.tensor.matmul(
                            pr0[:], ones[64:65, 0:64], rcb[64:65, 0:TOK],
                            start=True, stop=True,
                        )
                        nc.tensor.matmul(
                            pr1[:], ones[64:65, 0:64], rcb[64:65, TOK : 2 * TOK],
                            start=True, stop=True,
                        )
                        oh0s = p_ohs.tile([64, TOK], F8, tag="oh0s", name="oh0s")
                        oh1s = p_ohs.tile([64, TOK], F8, tag="oh1s", name="oh1s")
                        nc.vector.scalar_tensor_tensor(
                            oh0s[:], pob[0:64, 0:TOK], 1.0, pr0[:],
                            OP.mult, OP.mult,
                        )
                        nc.vector.scalar_tensor_tensor(
                            oh1s[:], pob[0:64, TOK : 2 * TOK], 1.0, pr1[:],
                            OP.mult, OP.mult,
                        )
                        nc.sync.dma_start(
                            send[128 * s_idx : 128 * s_idx + 64, :], oh0s[:]
                        )
                        nc.sync.dma_start(
                            send[128 * s_idx + 64 : 128 * (s_idx + 1), :],
                            oh1s[:],
                        )

        # ======== AllToAll: head shards -> token shards ========
        nc.gpsimd.collective_compute(
            "AllToAll", OP.bypass, replica_groups=GROUPS,
            ins=[send[:].opt()], outs=[recv[:].opt()],
        )

        # ======== Phases 3+4 scope: weight loads overlap the A2A ========
        with ExitStack() as hscope:
            p_w34 = hscope.enter_context(tc.tile_pool(name="w34", bufs=1))
            p_mid = hscope.enter_context(tc.tile_pool(name="mid", bufs=1))
            h3scope = ExitStack()
            p_hres = h3scope.enter_context(tc.tile_pool(name="hres", bufs=1))
            hres = [
                p_hres.tile([128, TOK], F32, tag=f"hres{k}", name=f"hres{k}")
                for k in range(KT)
            ]
            # proj + fc1 weights land in SBUF during the A2A wait
            wpa = p_w34.tile([128, KT * D], BF16, tag="wp", name="wp")
            nc.sync.dma_start(
                wpa[:].rearrange("p (k c) -> p k c", k=KT),
                wpT[:].rearrange("(k p) c -> p k c", p=128),
            )
            wpt = [wpa[:, D * k : D * (k + 1)] for k in range(KT)]
            w1a = []
            for g4 in range(4):
                w = p_w34.tile([128, 8 * D], BF16, tag=f"w1_{g4}", name=f"w1_{g4}")
                nc.sync.dma_start(
                    w[:], fc1T[:, 8 * D * g4 : 8 * D * (g4 + 1)]
                )
                w1a.append(w)
            w1 = [
                w1a[ht // 8][:, D * (ht % 8) : D * (ht % 8 + 1)]
                for ht in range(NH2)
            ]

            # Keep the PE busy through the A2A wait so HAM doesn't throttle
            # the clock down before proj: ~280 no-dep junk matmuls (N=128)
            # sit in the PE queue between the attention tail and proj.
            with tc.tile_pool(name="warm", bufs=1, space="PSUM") as psw:
                wt = psw.tile([128, 128], F32, tag="warm", name="warm")
                for _ in range(280):
                    nc.tensor.matmul(
                        wt[:], ones[:], ones[:], start=True, stop=True
                    )

            # residual slice LN (also overlaps the A2A)
            with (
                tc.tile_pool(name="res1", bufs=1) as p_res1,
                tc.tile_pool(name="stats1b", bufs=1) as p_stats1b,
                tc.tile_pool(name="fix1b", bufs=4) as p_fix1b,
                tc.tile_pool(name="ps1c", bufs=1, space="PSUM") as ps1c,
            ):
                xra = p_res1.tile([128, KT * TOK], F32, tag="xra", name="xra")
                nc.sync.dma_start(
                    xra[:].rearrange("p (k t) -> p k t", k=KT),
                    xres[:].rearrange("(k p) t -> p k t", p=128),
                )
                xrf = [xra[:, TOK * k : TOK * (k + 1)] for k in range(KT)]
                xrba = p_res1.tile([128, KT * TOK], BF16, tag="xrb", name="xrb")
                nc.vector.tensor_copy(xrba[:], xra[:])
                xrb = [xrba[:, TOK * k : TOK * (k + 1)] for k in range(KT)]
                psum_s = ps1c.tile([128, TOK], F32, tag="ps_s", name="ps_s")
                psum_q = ps1c.tile([128, TOK], F32, tag="ps_q", name="ps_q")
                for k in range(KT):
                    nc.tensor.matmul(
                        psum_s[:], ones[:], xrb[k][:],
                        start=(k == 0), stop=(k == KT - 1),
                    )
                for k in range(KT):
                    s = p_fix1b.tile([128, TOK], BF16, tag="sqr", name="sqr")
                    nc.vector.scalar_tensor_tensor(
                        s[:], xrb[k][:], 1.0, xrb[k][:], OP.mult, OP.mult
                    )
                    nc.tensor.matmul(
                        psum_q[:], ones[:], s[:],
                        start=(k == 0), stop=(k == KT - 1),
                    )
                a_r, b_r = _ln_stats(nc, p_stats1b, psum_s, psum_q, epst[:])
                for k in range(KT):
                    u = p_fix1b.tile([128, TOK], F32, tag="fixu", name="fixu")
                    nc.vector.scalar_tensor_tensor(
                        u[:], xrf[k][:], g1t[:, k : k + 1], a_r[:],
                        OP.mult, OP.mult,
                    )
                    nc.vector.scalar_tensor_tensor(
                        hres[k][:], b_r[:], g1t[:, k : k + 1], u[:],
                        OP.mult, OP.add,
                    )

            # ======== Phase 3: proj + residual + LN2 ========
            h2b, h2g = [], []
            with (
                tc.tile_pool(name="proj", bufs=1) as p_proj,
                tc.tile_pool(name="stats2", bufs=1) as p_stats2,
                tc.tile_pool(name="fix2", bufs=2) as p_fix2,
                tc.tile_pool(name="ps3", bufs=2, space="PSUM") as ps3,
                tc.tile_pool(name="ps3b", bufs=1, space="PSUM") as ps3b,
            ):
                of8 = p_proj.tile([128, KT * TOK], F8, tag="of8", name="of8")
                nc.sync.dma_start(
                    of8[:].rearrange("p (k t) -> p k t", k=KT),
                    recv[:].rearrange("(k p) t -> p k t", p=128),
                )
                ofa = p_proj.tile([128, KT * TOK], BF16, tag="ofa", name="ofa")
                nc.vector.tensor_copy(ofa[:], of8[:])
                ofull = [ofa[:, TOK * k : TOK * (k + 1)] for k in range(KT)]
                hrf, hrb = [], []
                psum_s = ps3b.tile([128, TOK], F32, tag="ps_s2", name="ps_s2")
                psum_q = ps3b.tile([128, TOK], F32, tag="ps_q2", name="ps_q2")
                for dt in range(KT):
                    pm = ps3.tile([128, TOK], F32, tag="ps_p", name="ps_p")
                    for k in range(KT):
                        nc.tensor.matmul(
                            pm[:], wpt[k][:, 128 * dt : 128 * (dt + 1)],
                            ofull[k],
                            start=(k == 0), stop=(k == KT - 1),
                        )
                    hf = p_proj.tile([128, TOK], F32, tag=f"hrf{dt}", name=f"hrf{dt}")
                    nc.vector.scalar_tensor_tensor(
                        hf[:], pm[:], 1.0, hres[dt][:], OP.mult, OP.add
                    )
                    hrf.append(hf)
                    hb = p_proj.tile([128, TOK], BF16, tag=f"hrb{dt}", name=f"hrb{dt}")
                    nc.vector.tensor_copy(hb[:], hf[:])
                    hrb.append(hb)
                    # interleave LN2 stats matmuls as each hrb tile lands
                    nc.tensor.matmul(
                        psum_s[:], ones[:], hb[:],
                        start=(dt == 0), stop=(dt == KT - 1),
                    )
                    s = p_fix2.tile([128, TOK], BF16, tag="sq2", name="sq2")
                    nc.scalar.activation(s[:], hb[:], AF.Square)
                    nc.tensor.matmul(
                        psum_q[:], ones[:], s[:],
                        start=(dt == 0), stop=(dt == KT - 1),
                    )
                a2, b2 = _ln_stats(nc, p_stats2, psum_s, psum_q, epst[:])
                for k in range(KT):
                    u = p_fix2.tile([128, TOK], F32, tag="fixu2", name="fixu2")
                    nc.vector.scalar_tensor_tensor(
                        u[:], hrf[k][:], 1.0, a2[:], OP.mult, OP.mult
                    )
                    h2f = p_fix2.tile([128, TOK], F32, tag="h2f", name="h2f")
                    nc.vector.scalar_tensor_tensor(
                        h2f[:], b2[:], 1.0, u[:], OP.mult, OP.add
                    )
                    hb = p_mid.tile([128, TOK], BF16, tag=f"h2b{k}", name=f"h2b{k}")
                    nc.vector.tensor_copy(hb[:], h2f[:])
                    h2b.append(hb)
                    hg = p_mid.tile([128, TOK], BF16, tag=f"h2g{k}", name=f"h2g{k}")
                    nc.scalar.mul(hg[:], h2f[:], g2t[:, k : k + 1])
                    h2g.append(hg)

            h3scope.close()

            # ======== Phase 4: MLP ========
            with (
                tc.tile_pool(name="m1p", bufs=1) as p_m1,
                tc.tile_pool(name="out4", bufs=3) as p_out4,
                tc.tile_pool(name="ps4", bufs=2, space="PSUM") as ps4,
                tc.tile_pool(name="ps4b", bufs=2, space="PSUM") as ps4b,
            ):
                m1 = []
                for hg in range(NH2 // 2):
                    pm4 = ps4.tile([128, 2 * TOK], F32, tag="ps_m1", name="ps_m1")
                    for sub in (0, 1):
                        ht = 2 * hg + sub
                        for k in range(KT):
                            nc.tensor.matmul(
                                pm4[:, TOK * sub : TOK * (sub + 1)],
                                w1[ht][:, 128 * k : 128 * (k + 1)], h2b[k][:],
                                start=(k == 0), stop=(k == KT - 1),
                            )
                    m = p_m1.tile(
                        [128, 2 * TOK], BF16, tag=f"m1_{hg}", name=f"m1_{hg}"
                    )
                    nc.scalar.activation(m[:], pm4[:], AF.Gelu)
                    m1.append(m)
                for dt in range(KT):
                    w2t = p_w34.tile(
                        [128, HID], BF16, tag="w2", name="w2", bufs=3
                    )
                    nc.sync.dma_start(
                        w2t[:], fc2T[:, HID * dt : HID * (dt + 1)]
                    )
                    pm = ps4b.tile([128, TOK], F32, tag="ps_f2", name="ps_f2")
                    for ht in range(NH2):
                        nc.tensor.matmul(
                            pm[:], w2t[:, 128 * ht : 128 * (ht + 1)],
                            m1[ht // 2][:, TOK * (ht % 2) : TOK * (ht % 2 + 1)],
                            start=(ht == 0), stop=(ht == NH2 - 1),
                        )
                    ot = p_out4.tile([128, TOK], F32, tag="otile", name="otile")
                    nc.vector.scalar_tensor_tensor(
                        ot[:], pm[:], 1.0, h2g[dt][:], OP.mult, OP.add
                    )
                    nc.sync.dma_start(outT[128 * dt : 128 * (dt + 1), :], ot[:])

    _split_multi_waits(nc)
    return nc


_CACHED_NC = None


def _get_program():
    global _CACHED_NC
    if _CACHED_NC is None:
        _CACHED_NC = build_program()
    return _CACHED_NC


def _prepare_in_maps(x, w_qkv, w_proj, w_fc1, w_fc2, g1, g2):
    bf = ml_dtypes.bfloat16
    x2 = np.ascontiguousarray(np.asarray(x, np.float32).reshape(T, D))
    xT_b = np.ascontiguousarray(x2.T).astype(bf)

    g1 = np.asarray(g1, np.float32)
    g2 = np.asarray(g2, np.float32)
    wqkv_g = np.asarray(w_qkv, np.float32) * g1[None, :]
    scale = HD ** -0.5
    wpT_b = np.ascontiguousarray(np.asarray(w_proj, np.float32).T).astype(bf)
    fc1g = np.asarray(w_fc1, np.float32) * g2[None, :]
    # fc1T[p, ht*1024 + k*128 + c] = fc1g[ht*128 + c, k*128 + p]
    fc1T_b = np.ascontiguousarray(
        fc1g.reshape(NH2, 128, KT, 128).transpose(3, 0, 2, 1).reshape(128, NH2 * D)
    ).astype(bf)
    # fc2T[p, dt*4096 + ht*128 + c] = w_fc2[dt*128 + c, ht*128 + p]
    fc2T_b = np.ascontiguousarray(
        np.asarray(w_fc2, np.float32)
        .reshape(KT, 128, NH2, 128)
        .transpose(3, 0, 2, 1)
        .reshape(128, KT * HID)
    ).astype(bf)
    ident = np.eye(128, dtype=np.float32).astype(bf)
    g1c = np.ascontiguousarray(g1.reshape(D, 1))
    g2c = np.ascontiguousarray(g2.reshape(D, 1))

    def rowsum_bf(w):
        return np.ascontiguousarray(
            w.astype(bf).astype(np.float32).sum(1).reshape(128, 1)
        )

    in_maps = []
    for c in range(N_CORES):
        rows = slice(128 * c, 128 * (c + 1))
        wq_c = wqkv_g[rows, :] * scale            # scale folded into q
        wk_c = wqkv_g[D : 2 * D][rows, :]
        wv_c = wqkv_g[2 * D :][rows, :]
        xres_c = np.ascontiguousarray(x2[TOK * c : TOK * (c + 1)].T)
        in_maps.append({
            "xT": xT_b,
            "xres": xres_c,
            "wqT": np.ascontiguousarray(wq_c.T).astype(bf),
            "wkT": np.ascontiguousarray(wk_c.T).astype(bf),
            "wvT": np.ascontiguousarray(wv_c.T).astype(bf),
            "wqs": rowsum_bf(wq_c),
            "wks": rowsum_bf(wk_c),
            "wvs": rowsum_bf(wv_c),
            "wpT": wpT_b,
            "fc1T": fc1T_b,
            "fc2T": fc2T_b,
            "g1c": g1c,
            "g2c": g2c,
            "ident": ident,
        })
    return in_maps


def run(inputs, trace=False, tmpdir=None):
    nc = _get_program()
    in_maps = _prepare_in_maps(**inputs)
    res = run_bass_kernel_spmd(
        nc, in_maps, list(range(N_CORES)), trace=trace, tmpdir=tmpdir
    )
    out = np.empty((T, D), np.float32)
    for c in range(N_CORES):
        out[TOK * c : TOK * (c + 1), :] = res.results[c]["outT"].T
    return out.reshape(B, L, D), res


def kernel(**inputs):
    out, _ = run(inputs, trace=False)
    return out


# revision 40
# speedup vs baseline: 1.0095x; 1.0095x over previous
"""Trainium2 Bass kernel for a dense transformer block (2x2048x1024, 16 heads,
MLP hidden 4096), SPMD over 8 NeuronCores.

Sharding: attention is head-sharded (2 heads per core, both batches); an
AllToAll converts head shards into token shards, after which proj/LN2/MLP run
on 512 tokens per core. All matmuls are bf16 with fp32 PSUM accumulation.
LayerNorm is folded into the QKV matmul epilogue: with P = W_g1 @ x^T,
qkv^T = P * istd + (-mu*istd) * rowsum(W_g1), so the normalized activations
are never materialized for the matmul path. Softmax skips max-subtraction
(scores are bounded ~|4| for this problem) and gets its denominator from an
appended ones-column in the AV matmul.

Optimizations over the original baseline:
- fc1/fc2/proj weights stored host-side in [128, N] layouts with 2KB
  contiguous partition lines and loaded with a handful of batched DMAs that
  overlap the AllToAll wait (wp+fc1 fully resident, fc2 triple-buffered).
- Attention scores land in 3-bank fp32 PSUM group tiles so each ACT exp
  instruction covers 1536 elements/lane; per-block softmax epilogues
  (reciprocal broadcast matmuls + scaling) are deferred to a tail after the
  whole attention loop so the PE instruction queue never stalls on the
  reciprocal chain and HAM stays un-throttled.
- LayerNorm istd uses exp(-0.5*ln(var+eps)) on the ACT engine, which shares
  one activation table set (natural_log_exp) with the softmax exp; squares
  ride half on ACT, half on DVE.
- The AllToAll payload is quantized to fp8e4m3 (error budget allows it) and
  widened back to bf16 on the receiving side; x-block/weight/const DMAs are
  batched into multi-dim single instructions to cut sync-engine issue time.
"""

from contextlib import ExitStack

import numpy as np
import ml_dtypes

import concourse.bass as bass
import concourse.mybir as mybir
from concourse import tile
from concourse.bass_utils import run_bass_kernel_spmd
from concourse.vector_clock import ScopedClock

F32 = mybir.dt.float32
BF16 = mybir.dt.bfloat16
F8 = mybir.dt.float8e4
AF = mybir.ActivationFunctionType
OP = mybir.AluOpType

N_CORES = 8
B, L, D = 2, 2048, 1024
NH, HD = 16, 64
HID = 4096
T = B * L            # 4096 tokens total
TOK = T // N_CORES   # 512 tokens per core after the A2A
KT = D // 128        # 8 k-tiles over the model dim
NJ = L // 128        # 16 j-tiles per batch
NI = L // TOK        # 4 i-blocks per batch
EPS = 1e-6
GROUPS = [list(range(N_CORES))]
NH2 = HID // 128     # 32 hidden tiles


class ChunkedDrainTileContext(tile.TileContext):
    """This walrus build only accepts one explicit sem wait per CTRL
    instruction; split the kernel-tail drain's waits across a chain."""

    MAX_WAITS = 1

    def _drain_and_barrier(self, tick_clock, wait_clock):
        drain_inst = self.nc.sync.drain()
        wait_clock.add_sem_waits(
            drain_inst.ins, ScopedClock({None: tick_clock.global_clock})
        )
        si = drain_inst.ins.sync_info
        if si is not None and len(si.on_wait) > self.MAX_WAITS:
            waits = list(si.on_wait)
            si.on_wait = waits[: self.MAX_WAITS]
            for i in range(self.MAX_WAITS, len(waits), self.MAX_WAITS):
                extra = self.nc.sync.drain()
                extra.ins.sync_info = mybir.SyncInfo(
                    on_wait=waits[i : i + self.MAX_WAITS], on_update=[]
                )
        self.nc.all_engine_barrier()
        assert self.sems is not None
        popped = self.nc._tile_sem_poison_stack.pop()
        assert popped is self._sem_poison
        self.nc.clear_and_free_semaphores(list(self.sems.allocated().values()))
        self.nc.all_engine_barrier()


def _split_multi_waits(nc):
    """This walrus build accepts at most one sync wait per instruction; hoist
    extra waits onto preceding same-engine NoOps."""
    n = 0
    for fn in nc.m.functions:
        for bb in fn.blocks:
            insts = bb.instructions
            new = []
            for ins in insts:
                si = ins.sync_info
                if si is not None and len(si.on_wait) > 1:
                    waits = list(si.on_wait)
                    si.on_wait = [waits[-1]]
                    for w in waits[:-1]:
                        n += 1
                        nop = mybir.InstNoOp(
                            name=f"waitsplit-{n}",
                            sync_info=mybir.SyncInfo(on_wait=[w], on_update=[]),
                            bass_nofuse=True,
                            engine=ins.engine,
                        )
                        nc.register_instruction(nop)
                        new.append(nop)
                new.append(ins)
            if len(new) != len(insts):
                bb.instructions = new
    return n


def _ln_stats(nc, pool, psum_s, psum_q, eps_ap):
    """From replicated column sums / sums-of-squares, produce replicated
    A = 1/std and B = -mu/std tiles, all [128, TOK] f32."""
    inv_d = 1.0 / D
    mu = pool.tile([128, TOK], F32, tag="mu", name="mu")
    nc.scalar.mul(mu[:], psum_s[:], inv_d)
    musq = pool.tile([128, TOK], F32, tag="musq", name="musq")
    nc.scalar.activation(musq[:], mu[:], AF.Square)
    var = pool.tile([128, TOK], F32, tag="var", name="var")
    nc.vector.scalar_tensor_tensor(
        var[:], psum_q[:], inv_d, musq[:], OP.mult, OP.subtract
    )
    # 1/sqrt(var+eps) = exp(-0.5*ln(var+eps)) -- keeps everything on ACT in
    # the same table set as the softmax exp (natural_log_exp_and_others).
    sv = pool.tile([128, TOK], F32, tag="sv", name="sv")
    nc.scalar.activation(sv[:], var[:], AF.Ln, bias=eps_ap)
    a_t = pool.tile([128, TOK], F32, tag="a_t", name="a_t")
    nc.scalar.activation(a_t[:], sv[:], AF.Exp, scale=-0.5)
    b_t = pool.tile([128, TOK], F32, tag="b_t", name="b_t")
    nc.vector.scalar_tensor_tensor(b_t[:], mu[:], -1.0, a_t[:], OP.mult, OP.mult)
    return a_t, b_t


def build_program():
    nc = bass.Bass(
        "TRN2", target_bir_lowering=False, debug=False, num_devices=N_CORES
    )

    xT = nc.dram_tensor("xT", [D, T], BF16, kind="ExternalInput")
    xres = nc.dram_tensor("xres", [D, TOK], F32, kind="ExternalInput")
    wqT = nc.dram_tensor("wqT", [D, 128], BF16, kind="ExternalInput")
    wkT = nc.dram_tensor("wkT", [D, 128], BF16, kind="ExternalInput")
    wvT = nc.dram_tensor("wvT", [D, 128], BF16, kind="ExternalInput")
    wqs = nc.dram_tensor("wqs", [128, 1], F32, kind="ExternalInput")
    wks = nc.dram_tensor("wks", [128, 1], F32, kind="ExternalInput")
    wvs = nc.dram_tensor("wvs", [128, 1], F32, kind="ExternalInput")
    wpT = nc.dram_tensor("wpT", [D, D], BF16, kind="ExternalInput")
    fc1T = nc.dram_tensor("fc1T", [128, NH2 * D], BF16, kind="ExternalInput")
    fc2T = nc.dram_tensor("fc2T", [128, KT * HID], BF16, kind="ExternalInput")
    g1c = nc.dram_tensor("g1c", [D, 1], F32, kind="ExternalInput")
    g2c = nc.dram_tensor("g2c", [D, 1], F32, kind="ExternalInput")
    ident = nc.dram_tensor("ident", [128, 128], BF16, kind="ExternalInput")
    outT = nc.dram_tensor("outT", [D, TOK], F32, kind="ExternalOutput")

    with ChunkedDrainTileContext(nc) as tc, ExitStack() as outer:
        p_const = outer.enter_context(tc.tile_pool(name="const", bufs=1))
        p_dram = outer.enter_context(tc.tile_pool(name="dram", bufs=1, space="DRAM"))

        ones = p_const.tile([128, 128], BF16, tag="ones", name="ones")
        nc.gpsimd.memset(ones[:], 1.0)
        idt = p_const.tile([128, 128], BF16, tag="idt", name="idt")
        g1t = p_const.tile([128, KT], F32, tag="g1t", name="g1t")
        g2t = p_const.tile([128, KT], F32, tag="g2t", name="g2t")
        epst = p_const.tile([128, 1], F32, tag="epst", name="epst")
        nc.gpsimd.memset(epst[:], EPS)

        def load_consts():
            # deferred so the first x-block DMA wins the sync queue at startup
            nc.sync.dma_start(idt[:], ident[:])
            nc.sync.dma_start(
                g1t[:].rearrange("p (k c) -> p k c", k=KT),
                g1c[:].rearrange("(k p) c -> p k c", p=128),
            )
            nc.sync.dma_start(
                g2t[:].rearrange("p (k c) -> p k c", k=KT),
                g2c[:].rearrange("(k p) c -> p k c", p=128),
            )

        send = p_dram.tile([T // 4, TOK], F8, tag="send", name="send")
        recv = p_dram.tile([T // 4, TOK], F8, tag="recv", name="recv")

        with ExitStack() as qscope:
            p_qkvT = qscope.enter_context(tc.tile_pool(name="qkvT", bufs=1))
            qTt = p_qkvT.tile([128, T], BF16, tag="qT", name="qT")
            kTt = p_qkvT.tile([128, T], BF16, tag="kT", name="kT")
            vTt = p_qkvT.tile([128, T], BF16, tag="vT", name="vT")

            # ======== Phase 1: LN1 stats + QKV ========
            with (
                tc.tile_pool(name="xtb", bufs=3) as p_xtb,
                tc.tile_pool(name="wqkv", bufs=1) as p_wqkv,
                tc.tile_pool(name="sq", bufs=2) as p_sq,
                tc.tile_pool(name="stats", bufs=2) as p_stats,
                tc.tile_pool(name="fix", bufs=4) as p_fix,
                tc.tile_pool(name="ps1", bufs=2, space="PSUM") as ps1,
                tc.tile_pool(name="ps1b", bufs=2, space="PSUM") as ps1b,
            ):
                xa_tiles = {}

                def load_xa(tb):
                    sl = slice(TOK * tb, TOK * (tb + 1))
                    # one batched DMA per token block: [128, KT*TOK] where
                    # column group k holds xT[128k:128(k+1), sl]
                    xa = p_xtb.tile([128, KT * TOK], BF16, tag="xtb", name="xtb")
                    nc.sync.dma_start(
                        xa[:].rearrange("p (k t) -> p k t", k=KT),
                        xT[:, sl].rearrange("(k p) t -> p k t", p=128),
                    )
                    xa_tiles[tb] = xa

                load_xa(0)
                wqkv = []
                for src, name in ((wqT, "wq"), (wkT, "wk"), (wvT, "wv")):
                    t = p_wqkv.tile([128, D], BF16, tag=name, name=name)
                    nc.sync.dma_start(
                        t[:].rearrange("p (k c) -> p k c", k=KT),
                        src[:].rearrange("(k p) c -> p k c", p=128),
                    )
                    wqkv.append(t)
                wqsum = p_wqkv.tile([128, 1], F32, tag="wqsum", name="wqsum")
                nc.sync.dma_start(wqsum[:], wqs[:])
                wksum = p_wqkv.tile([128, 1], F32, tag="wksum", name="wksum")
                nc.sync.dma_start(wksum[:], wks[:])
                wvsum = p_wqkv.tile([128, 1], F32, tag="wvsum", name="wvsum")
                nc.sync.dma_start(wvsum[:], wvs[:])
                load_xa(1)
                load_consts()

                for tb in range(T // TOK):
                    sl = slice(TOK * tb, TOK * (tb + 1))
                    if tb + 2 < T // TOK:
                        load_xa(tb + 2)
                    xa = xa_tiles.pop(tb)
                    xtb = [xa[:, TOK * k : TOK * (k + 1)] for k in range(KT)]
                    sqa = p_sq.tile([128, KT * TOK], BF16, tag="sq", name="sq")
                    half = KT * TOK // 2
                    nc.scalar.activation(sqa[:, 0:half], xa[:, 0:half], AF.Square)
                    nc.vector.scalar_tensor_tensor(
                        sqa[:, half:], xa[:, half:], 1.0, xa[:, half:],
                        OP.mult, OP.mult,
                    )
                    sq = [sqa[:, TOK * k : TOK * (k + 1)] for k in range(KT)]
                    psum_s = ps1.tile([128, TOK], F32, tag="ps_s", name="ps_s")
                    psum_q = ps1.tile([128, TOK], F32, tag="ps_q", name="ps_q")
                    for k in range(KT):
                        nc.tensor.matmul(
                            psum_s[:], ones[:], xtb[k],
                            start=(k == 0), stop=(k == KT - 1),
                        )
                    for k in range(KT):
                        nc.tensor.matmul(
                            psum_q[:], ones[:], sq[k],
                            start=(k == 0), stop=(k == KT - 1),
                        )
                    a_t, b_t = _ln_stats(nc, p_stats, psum_s, psum_q, epst[:])

                    for dst, w, wsum in (
                        (qTt, wqkv[0], wqsum),
                        (kTt, wqkv[1], wksum),
                        (vTt, wqkv[2], wvsum),
                    ):
                        pm = ps1b.tile([128, TOK], F32, tag="ps_qkv", name="ps_qkv")
                        for k in range(KT):
                            nc.tensor.matmul(
                                pm[:], w[:, 128 * k : 128 * (k + 1)], xtb[k],
                                start=(k == 0), stop=(k == KT - 1),
                            )
                        u = p_fix.tile([128, TOK], F32, tag="fixu", name="fixu")
                        nc.vector.scalar_tensor_tensor(
                            u[:], pm[:], 1.0, a_t[:], OP.mult, OP.mult
                        )
                        nc.vector.scalar_tensor_tensor(
                            dst[:, sl], b_t[:], wsum[:], u[:], OP.mult, OP.add
                        )

            # ======== Phase 2: attention ========
            with (
                tc.tile_pool(name="vones", bufs=1) as p_vones,
                tc.tile_pool(name="es", bufs=4) as p_es,
                tc.tile_pool(name="attn", bufs=3) as p_attn,
                tc.tile_pool(name="ohs", bufs=2) as p_ohs,
            ):
                vones = []
                with tc.tile_pool(name="psv", bufs=2, space="PSUM") as psv:
                    for t in range(T // 128):
                        pm = psv.tile([128, 128], BF16, tag="ps_vt", name="ps_vt")
                        nc.tensor.transpose(
                            pm[:], vTt[:, 128 * t : 128 * (t + 1)], idt[:]
                        )
                        vo = p_vones.tile(
                            [128, 130], BF16, tag=f"vo{t}", name=f"vo{t}"
                        )
                        nc.gpsimd.memset(vo[:], 1.0)
                        nc.vector.tensor_copy(vo[:, 0:64], pm[:, 0:64])
                        nc.vector.tensor_copy(vo[:, 65:129], pm[:, 64:128])
                        vones.append(vo)

                finished = []
                with (
                    tc.tile_pool(name="pss", bufs=2, space="PSUM") as pss,
                    tc.tile_pool(name="pso", bufs=1, space="PSUM") as pso,
                ):
                    for b in range(B):
                        for i in range(NI):
                            isl = slice(b * L + TOK * i, b * L + TOK * (i + 1))
                            po0 = pso.tile([65, TOK], F32, tag="po0", name="po0")
                            po1 = pso.tile([65, TOK], F32, tag="po1", name="po1")
                            po = (po0, po1)
                            # 32 score tiles (j-major, head-minor), exp'd in
                            # groups of 3 from a 3-bank PSUM tile
                            for g in range(11):
                                nslot = 3 if g < 10 else 2
                                sg = pss.tile(
                                    [128, 1536], F32, tag="sg", name="sg"
                                )
                                for u_ in range(nslot):
                                    s = 3 * g + u_
                                    j, h = s // 2, s % 2
                                    jsl = slice(
                                        b * L + 128 * j, b * L + 128 * (j + 1)
                                    )
                                    nc.tensor.matmul(
                                        sg[:, 512 * u_ : 512 * (u_ + 1)],
                                        kTt[64 * h : 64 * (h + 1), jsl],
                                        qTt[64 * h : 64 * (h + 1), isl],
                                        start=True, stop=True,
                                        tile_position=(64 * h, 0),
                                    )
                                es = p_es.tile(
                                    [128, 1536], BF16, tag="es", name="es"
                                )
                                nc.scalar.activation(
                                    es[:, 0 : 512 * nslot], sg[:, 0 : 512 * nslot],
                                    AF.Exp,
                                )
                                for u_ in range(nslot):
                                    s = 3 * g + u_
                                    j, h = s // 2, s % 2
                                    vo = vones[b * NJ + j]
                                    nc.tensor.matmul(
                                        po[h][:],
                                        vo[:, 65 * h : 65 * (h + 1)],
                                        es[:, 512 * u_ : 512 * (u_ + 1)],
                                        start=(s == h), stop=(s == 30 + h),
                                    )
                            # copy AV results to SBUF promptly (frees the po
                            # PSUM banks for the next block's accumulation)
                            pob = p_attn.tile(
                                [65, 2 * TOK], F32, tag="pob", name="pob", bufs=8
                            )
                            nc.vector.tensor_copy(pob[:, 0:TOK], po0[:])
                            nc.vector.tensor_copy(pob[:, TOK : 2 * TOK], po1[:])
                            # softmax denominators sit in row 64
                            rc = p_attn.tile(
                                [65, 2 * TOK], F32, tag="rc", name="rc", bufs=2
                            )
                            nc.vector.reciprocal(rc[64:65, :], pob[64:65, :])
                            rcb = p_attn.tile(
                                [65, 2 * TOK], BF16, tag="rcb", name="rcb", bufs=8
                            )
                            nc.vector.tensor_copy(rcb[64:65, :], rc[64:65, :])
                            finished.append((pob, rcb, b * NI + i))

                # epilogue tail: broadcast 1/denominator across partitions via
                # PE, scale, and ship each block to its destination core
                with tc.tile_pool(name="psr", bufs=1, space="PSUM") as psr:
                    for pob, rcb, s_idx in finished:
                        pr0 = psr.tile([64, TOK], F32, tag="pr0", name="pr0")
                        pr1 = psr.tile([64, TOK], F32, tag="pr1", name="pr1")
                        nc.tensor.matmul(
                            pr0[:], ones[64:65, 0:64], rcb[64:65, 0:TOK],
                            start=True, stop=True,
                        )
                        nc.tensor.matmul(
                            pr1[:], ones[64:65, 0:64], rcb[64:65, TOK : 2 * TOK],
                            start=True, stop=True,
                        )
                        oh0s = p_ohs.tile([64, TOK], F8, tag="oh0s", name="oh0s")
                        oh1s = p_ohs.tile([64, TOK], F8, tag="oh1s", name="oh1s")
                        nc.vector.scalar_tensor_tensor(
                            oh0s[:], pob[0:64, 0:TOK], 1.0, pr0[:],
                            OP.mult, OP.mult,
                        )
                        nc.vector.scalar_tensor_tensor(
                            oh1s[:], pob[0:64, TOK : 2 * TOK], 1.0, pr1[:],
                            OP.mult, OP.mult,
                        )
                        nc.sync.dma_start(
                            send[128 * s_idx : 128 * s_idx + 64, :], oh0s[:]
                        )
                        nc.sync.dma_start(
                            send[128 * s_idx + 64 : 128 * (s_idx + 1), :],
                            oh1s[:],
                        )

        # ======== AllToAll: head shards -> token shards ========
        nc.gpsimd.collective_compute(
            "AllToAll", OP.bypass, replica_groups=GROUPS,
            ins=[send[:].opt()], outs=[recv[:].opt()],
        )

        # ======== Phases 3+4 scope: weight loads overlap the A2A ========
        with ExitStack() as hscope:
            p_w34 = hscope.enter_context(tc.tile_pool(name="w34", bufs=1))
            p_mid = hscope.enter_context(tc.tile_pool(name="mid", bufs=1))
            h3scope = ExitStack()
            p_hres = h3scope.enter_context(tc.tile_pool(name="hres", bufs=1))
            hres = [
                p_hres.tile([128, TOK], F32, tag=f"hres{k}", name=f"hres{k}")
                for k in range(KT)
            ]
            # proj + fc1 weights land in SBUF during the A2A wait
            wpa = p_w34.tile([128, KT * D], BF16, tag="wp", name="wp")
            nc.sync.dma_start(
                wpa[:].rearrange("p (k c) -> p k c", k=KT),
                wpT[:].rearrange("(k p) c -> p k c", p=128),
            )
            wpt = [wpa[:, D * k : D * (k + 1)] for k in range(KT)]
            w1a = []
            for g4 in range(4):
                w = p_w34.tile([128, 8 * D], BF16, tag=f"w1_{g4}", name=f"w1_{g4}")
                nc.sync.dma_start(
                    w[:], fc1T[:, 8 * D * g4 : 8 * D * (g4 + 1)]
                )
                w1a.append(w)
            w1 = [
                w1a[ht // 8][:, D * (ht % 8) : D * (ht % 8 + 1)]
                for ht in range(NH2)
            ]

            # residual slice LN (also overlaps the A2A)
            with (
                tc.tile_pool(name="res1", bufs=1) as p_res1,
                tc.tile_pool(name="stats1b", bufs=1) as p_stats1b,
                tc.tile_pool(name="fix1b", bufs=4) as p_fix1b,
                tc.tile_pool(name="ps1c", bufs=1, space="PSUM") as ps1c,
            ):
                xra = p_res1.tile([128, KT * TOK], F32, tag="xra", name="xra")
                nc.sync.dma_start(
                    xra[:].rearrange("p (k t) -> p k t", k=KT),
                    xres[:].rearrange("(k p) t -> p k t", p=128),
                )
                xrf = [xra[:, TOK * k : TOK * (k + 1)] for k in range(KT)]
                xrba = p_res1.tile([128, KT * TOK], BF16, tag="xrb", name="xrb")
                nc.vector.tensor_copy(xrba[:], xra[:])
                xrb = [xrba[:, TOK * k : TOK * (k + 1)] for k in range(KT)]
                psum_s = ps1c.tile([128, TOK], F32, tag="ps_s", name="ps_s")
                psum_q = ps1c.tile([128, TOK], F32, tag="ps_q", name="ps_q")
                for k in range(KT):
                    nc.tensor.matmul(
                        psum_s[:], ones[:], xrb[k][:],
                        start=(k == 0), stop=(k == KT - 1),
                    )
                for k in range(KT):
                    s = p_fix1b.tile([128, TOK], BF16, tag="sqr", name="sqr")
                    nc.vector.scalar_tensor_tensor(
                        s[:], xrb[k][:], 1.0, xrb[k][:], OP.mult, OP.mult
                    )
                    nc.tensor.matmul(
                        psum_q[:], ones[:], s[:],
                        start=(k == 0), stop=(k == KT - 1),
                    )
                a_r, b_r = _ln_stats(nc, p_stats1b, psum_s, psum_q, epst[:])
                for k in range(KT):
                    u = p_fix1b.tile([128, TOK], F32, tag="fixu", name="fixu")
                    nc.vector.scalar_tensor_tensor(
                        u[:], xrf[k][:], g1t[:, k : k + 1], a_r[:],
                        OP.mult, OP.mult,
                    )
                    nc.vector.scalar_tensor_tensor(
                        hres[k][:], b_r[:], g1t[:, k : k + 1], u[:],
                        OP.mult, OP.add,
                    )

            # ======== Phase 3: proj + residual + LN2 ========
            h2b, h2g = [], []
            with (
                tc.tile_pool(name="proj", bufs=1) as p_proj,
                tc.tile_pool(name="stats2", bufs=1) as p_stats2,
                tc.tile_pool(name="fix2", bufs=2) as p_fix2,
                tc.tile_pool(name="ps3", bufs=2, space="PSUM") as ps3,
                tc.tile_pool(name="ps3b", bufs=1, space="PSUM") as ps3b,
            ):
                of8 = p_proj.tile([128, KT * TOK], F8, tag="of8", name="of8")
                nc.sync.dma_start(
                    of8[:].rearrange("p (k t) -> p k t", k=KT),
                    recv[:].rearrange("(k p) t -> p k t", p=128),
                )
                ofa = p_proj.tile([128, KT * TOK], BF16, tag="ofa", name="ofa")
                nc.vector.tensor_copy(ofa[:], of8[:])
                ofull = [ofa[:, TOK * k : TOK * (k + 1)] for k in range(KT)]
                hrf, hrb = [], []
                psum_s = ps3b.tile([128, TOK], F32, tag="ps_s2", name="ps_s2")
                psum_q = ps3b.tile([128, TOK], F32, tag="ps_q2", name="ps_q2")
                for dt in range(KT):
                    pm = ps3.tile([128, TOK], F32, tag="ps_p", name="ps_p")
                    for k in range(KT):
                        nc.tensor.matmul(
                            pm[:], wpt[k][:, 128 * dt : 128 * (dt + 1)],
                            ofull[k],
                            start=(k == 0), stop=(k == KT - 1),
                        )
                    hf = p_proj.tile([128, TOK], F32, tag=f"hrf{dt}", name=f"hrf{dt}")
                    nc.vector.scalar_tensor_tensor(
                        hf[:], pm[:], 1.0, hres[dt][:], OP.mult, OP.add
                    )
                    hrf.append(hf)
                    hb = p_proj.tile([128, TOK], BF16, tag=f"hrb{dt}", name=f"hrb{dt}")
                    nc.vector.tensor_copy(hb[:], hf[:])
                    hrb.append(hb)
                    # interleave LN2 stats matmuls as each hrb tile lands
                    nc.tensor.matmul(
                        psum_s[:], ones[:], hb[:],
                        start=(dt == 0), stop=(dt == KT - 1),
                    )
                    s = p_fix2.tile([128, TOK], BF16, tag="sq2", name="sq2")
                    nc.scalar.activation(s[:], hb[:], AF.Square)
                    nc.tensor.matmul(
                        psum_q[:], ones[:], s[:],
                        start=(dt == 0), stop=(dt == KT - 1),
                    )
                a2, b2 = _ln_stats(nc, p_stats2, psum_s, psum_q, epst[:])
                for k in range(KT):
                    u = p_fix2.tile([128, TOK], F32, tag="fixu2", name="fixu2")
                    nc.vector.scalar_tensor_tensor(
                        u[:], hrf[k][:], 1.0, a2[:], OP.mult, OP.mult
                    )
                    h2f = p_fix2.tile([128, TOK], F32, tag="h2f", name="h2f")
                    nc.vector.scalar_tensor_tensor(
                        h2f[:], b2[:], 1.0, u[:], OP.mult, OP.add
                    )
                    hb = p_mid.tile([128, TOK], BF16, tag=f"h2b{k}", name=f"h2b{k}")
                    nc.vector.tensor_copy(hb[:], h2f[:])
                    h2b.append(hb)
                    hg = p_mid.tile([128, TOK], BF16, tag=f"h2g{k}", name=f"h2g{k}")
                    nc.scalar.mul(hg[:], h2f[:], g2t[:, k : k + 1])
                    h2g.append(hg)

            h3scope.close()

            # ======== Phase 4: MLP ========
            with (
                tc.tile_pool(name="m1p", bufs=1) as p_m1,
                tc.tile_pool(name="out4", bufs=3) as p_out4,
                tc.tile_pool(name="ps4", bufs=2, space="PSUM") as ps4,
                tc.tile_pool(name="ps4b", bufs=2, space="PSUM") as ps4b,
            ):
                m1 = []
                for hg in range(NH2 // 2):
                    pm4 = ps4.tile([128, 2 * TOK], F32, tag="ps_m1", name="ps_m1")
                    for sub in (0, 1):
                        ht = 2 * hg + sub
                        for k in range(KT):
                            nc.tensor.matmul(
                                pm4[:, TOK * sub : TOK * (sub + 1)],
                                w1[ht][:, 128 * k : 128 * (k + 1)], h2b[k][:],
                                start=(k == 0), stop=(k == KT - 1),
                            )
                    m = p_m1.tile(
                        [128, 2 * TOK], BF16, tag=f"m1_{hg}", name=f"m1_{hg}"
                    )
                    nc.scalar.activation(m[:], pm4[:], AF.Gelu)
                    m1.append(m)
                for dt in range(KT):
                    w2t = p_w34.tile(
                        [128, HID], BF16, tag="w2", name="w2", bufs=3
                    )
                    nc.sync.dma_start(
                        w2t[:], fc2T[:, HID * dt : HID * (dt + 1)]
                    )
                    pm = ps4b.tile([128, TOK], F32, tag="ps_f2", name="ps_f2")
                    for ht in range(NH2):
                        nc.tensor.matmul(
                            pm[:], w2t[:, 128 * ht : 128 * (ht + 1)],
                            m1[ht // 2][:, TOK * (ht % 2) : TOK * (ht % 2 + 1)],
                            start=(ht == 0), stop=(ht == NH2 - 1),
                        )
                    ot = p_out4.tile([128, TOK], F32, tag="otile", name="otile")
                    nc.vector.scalar_tensor_tensor(
                        ot[:], pm[:], 1.0, h2g[dt][:], OP.mult, OP.add
                    )
                    nc.sync.dma_start(outT[128 * dt : 128 * (dt + 1), :], ot[:])

    _split_multi_waits(nc)
    return nc


_CACHED_NC = None


def _get_program():
    global _CACHED_NC
    if _CACHED_NC is None:
        _CACHED_NC = build_program()
    return _CACHED_NC


def _prepare_in_maps(x, w_qkv, w_proj, w_fc1, w_fc2, g1, g2):
    bf = ml_dtypes.bfloat16
    x2 = np.ascontiguousarray(np.asarray(x, np.float32).reshape(T, D))
    xT_b = np.ascontiguousarray(x2.T).astype(bf)

    g1 = np.asarray(g1, np.float32)
    g2 = np.asarray(g2, np.float32)
    wqkv_g = np.asarray(w_qkv, np.float32) * g1[None, :]
    scale = HD ** -0.5
    wpT_b = np.ascontiguousarray(np.asarray(w_proj, np.float32).T).astype(bf)
    fc1g = np.asarray(w_fc1, np.float32) * g2[None, :]
    # fc1T[p, ht*1024 + k*128 + c] = fc1g[ht*128 + c, k*128 + p]
    fc1T_b = np.ascontiguousarray(
        fc1g.reshape(NH2, 128, KT, 128).transpose(3, 0, 2, 1).reshape(128, NH2 * D)
    ).astype(bf)
    # fc2T[p, dt*4096 + ht*128 + c] = w_fc2[dt*128 + c, ht*128 + p]
    fc2T_b = np.ascontiguousarray(
        np.asarray(w_fc2, np.float32)
        .reshape(KT, 128, NH2, 128)
        .transpose(3, 0, 2, 1)
        .reshape(128, KT * HID)
    ).astype(bf)
    ident = np.eye(128, dtype=np.float32).astype(bf)
    g1c = np.ascontiguousarray(g1.reshape(D, 1))
    g2c = np.ascontiguousarray(g2.reshape(D, 1))

    def rowsum_bf(w):
        return np.ascontiguousarray(
            w.astype(bf).astype(np.float32).sum(1).reshape(128, 1)
        )

    in_maps = []
    for c in range(N_CORES):
        rows = slice(128 * c, 128 * (c + 1))
        wq_c = wqkv_g[rows, :] * scale            # scale folded into q
        wk_c = wqkv_g[D : 2 * D][rows, :]
        wv_c = wqkv_g[2 * D :][rows, :]
        xres_c = np.ascontiguousarray(x2[TOK * c : TOK * (c + 1)].T)
        in_maps.append({
            "xT": xT_b,
            "xres": xres_c,
            "wqT": np.ascontiguousarray(wq_c.T).astype(bf),
            "wkT": np.ascontiguousarray(wk_c.T).astype(bf),
            "wvT": np.ascontiguousarray(wv_c.T).astype(bf),
            "wqs": rowsum_bf(wq_c),
            "wks": rowsum_bf(wk_c),
            "wvs": rowsum_bf(wv_c),
            "wpT": wpT_b,
            "fc1T": fc1T_b,
            "fc2T": fc2T_b,
            "g1c": g1c,
            "g2c": g2c,
            "ident": ident,
        })
    return in_maps


def run(inputs, trace=False, tmpdir=None):
    nc = _get_program()
    in_maps = _prepare_in_maps(**inputs)
    res = run_bass_kernel_spmd(
        nc, in_maps, list(range(N_CORES)), trace=trace, tmpdir=tmpdir
    )
    out = np.empty((T, D), np.float32)
    for c in range(N_CORES):
        out[TOK * c : TOK * (c + 1), :] = res.results[c]["outT"].T
    return out.reshape(B, L, D), res


def kernel(**inputs):
    out, _ = run(inputs, trace=False)
    return out
